# revision 6
# baseline (speedup 1.0000x reference)
"""Trainium2 Bass kernel for nn_CompetitiveLayer (competitive binding equilibrium).

reference:
    K = exp(k); BT = exp(bt); BTb = broadcast(BT, (B, nB))
    AF, BF = AT, BTb
    repeat N_ITERS times:
        AF = AT  / (BF @ K.T + 1)
        BF = BTb / (AF @ K   + 1)
    AF_ = AT  / (BF @ K.T + 1)
    BF_ = BTb / (AF @ K   + 1)        # == BF from the last iteration
    C = AF_[:, :, None] * BF_[:, None, :] * K     # [B, nA, nB]

Strategy (pure data parallel over 8 cores, 65536 rows each):
  - "grouped" on-chip layout: 128 partitions = 16 groups x 8 (the nA/nB dim),
    batch along the free dim.  One column holds 16 independent rows.
  - the per-row 8x8 matvecs become ONE 128-contraction matmul with a
    block-diagonal weight matrix (16 copies of K on the diagonal).
  - we track BF' = BF / BT, so both elementwise steps are pure 1/(1+x):
        u = BF' @ (diag(BT) K).T     AF  = AT * recip(1+u)
        v = AF @ K                   BF' = recip(1+v)
  - reciprocals: exp(-ln(1+x)) on the Scalar engine (Ln fuses the +1 via its
    bias operand, reading the matmul PSUM directly), and/or DVE ops.
  - output outer product: two "spread" matmuls (stationary = state block,
    moving = 0/1*K pattern) produce D[b,(i,j)]=AF_[b,i] and
    E[b,(i,j)]=K[i,j]*BT[j]*BF'_[b,j] in batch-major layout; C = D*E is one
    tensor_tensor multiply, and the C tile maps to a fully contiguous 2MB DMA.
"""

import os
import sys
import functools

import numpy as np

for _p in ("/opt/trn_rl_repo", "/root/.axon_site/_ro/trn_rl_repo"):
    if os.path.isdir(_p) and _p not in sys.path:
        sys.path.insert(0, _p)

import concourse.bass as bass
import concourse.bacc as bacc
import concourse.tile as tile
import concourse.mybir as mybir
from concourse import bass_utils

F32 = mybir.dt.float32
AFT = mybir.ActivationFunctionType

N_CORES = 8
B_TOTAL = 524288
NA = 8
NB = 8
ROWS_PER_CORE = B_TOTAL // N_CORES        # 65536
CHUNK_ROWS = 8192                         # rows per on-chip chunk
N_ITERS = 20                              # fixed-point iterations (reference: 20)

# Matmul dtype for the iteration / spread matmuls. float32 is exact but runs
# at 1/4 rate on the PE; float32r runs at full rate (free dim >= 256).
MM_DT = F32


def _block_diag16(block8: np.ndarray) -> np.ndarray:
    """[8,8] -> [128,128] with 16 copies on the diagonal."""
    W = np.zeros((128, 128), np.float32)
    for g in range(16):
        W[g * 8:(g + 1) * 8, g * 8:(g + 1) * 8] = block8
    return W


def _host_weights(k: np.ndarray, bt: np.ndarray) -> dict[str, np.ndarray]:
    K = np.exp(k.astype(np.float64)).astype(np.float32)          # [8,8]
    BT = np.exp(bt.astype(np.float64)).astype(np.float32).reshape(-1)  # [8]

    # u-matmul: U[(g,i), col] = sum_j Wu[(g,j),(g,i)] * BF'[(g,j), col]
    #           need BT_j * K[i,j]  ->  block[j, i] = BT[j] * K[i, j]
    Wu = _block_diag16((BT[:, None] * K.T))
    # v-matmul: V[(g,j), col] = sum_i Wv[(g,i),(g,j)] * AF[(g,i), col]
    #           block[i, j] = K[i, j]
    Wv = _block_diag16(K)

    # spread matmuls (moving operand), out free index n = (g - 8h)*64 + i*8 + j
    # D: out[w, n] = AF_[(g,i), w]        -> W2D[(g,i'), n] = delta_{i,i'}
    # E: out[w, n] = K[i,j]*BT[j]*BF'[(g,j), w]
    W2D = np.zeros((128, 1024), np.float32)
    W2E = np.zeros((128, 1024), np.float32)
    for g in range(16):
        h, gl = divmod(g, 8)
        for i in range(8):
            for j in range(8):
                n = h * 512 + gl * 64 + i * 8 + j
                W2D[g * 8 + i, n] = 1.0
                W2E[g * 8 + j, n] = K[i, j] * BT[j]

    IDT = np.eye(128, dtype=np.float32)
    return {"Wu": Wu, "Wv": Wv, "W2D": W2D, "W2E": W2E, "IDT": IDT}


def _build_program(rows_per_core: int, n_iters: int):
    """Build the single-core Bass program (SPMD-replicated across cores)."""
    n_chunks = rows_per_core // CHUNK_ROWS
    assert rows_per_core % CHUNK_ROWS == 0
    W = CHUNK_ROWS // 64                  # batch-major free cols per s-block = 128
    assert CHUNK_ROWS % 64 == 0 and W == 128
    SFREE = 4 * W                         # grouped free size per chunk = 512

    nc = bacc.Bacc("TRN2", target_bir_lowering=False, debug=False)

    at_d = nc.dram_tensor("AT", [rows_per_core, NA], F32, kind="ExternalInput")
    wu_d = nc.dram_tensor("Wu", [128, 128], F32, kind="ExternalInput")
    wv_d = nc.dram_tensor("Wv", [128, 128], F32, kind="ExternalInput")
    w2d_d = nc.dram_tensor("W2D", [128, 1024], F32, kind="ExternalInput")
    w2e_d = nc.dram_tensor("W2E", [128, 1024], F32, kind="ExternalInput")
    idt_d = nc.dram_tensor("IDT", [128, 128], F32, kind="ExternalInput")
    c_d = nc.dram_tensor("C", [rows_per_core, NA * NB], F32, kind="ExternalOutput")

    at_chunks = at_d.ap().rearrange("(c p t) i -> c p (t i)", c=n_chunks, p=128)
    c_chunks = c_d.ap().rearrange("(c p t) n -> c p (t n)", c=n_chunks, p=128)

    with tile.TileContext(nc) as tc:
        with (
            tc.tile_pool(name="wgt", bufs=1) as wgt,
            tc.tile_pool(name="io", bufs=2) as io,
            tc.tile_pool(name="state", bufs=2) as state,
            tc.tile_pool(name="tps", bufs=2, space="PSUM") as tps,
            tc.tile_pool(name="uvps", bufs=2, space="PSUM") as uvps,
            tc.tile_pool(name="deps", bufs=4, space="PSUM") as deps,
        ):
            wu = wgt.tile([128, 128], F32)
            nc.sync.dma_start(wu[:], wu_d.ap())
            wv = wgt.tile([128, 128], F32)
            nc.sync.dma_start(wv[:], wv_d.ap())
            w2d = wgt.tile([128, 1024], F32)
            nc.sync.dma_start(w2d[:], w2d_d.ap())
            w2e = wgt.tile([128, 1024], F32)
            nc.sync.dma_start(w2e[:], w2e_d.ap())
            idt = wgt.tile([128, 128], F32)
            nc.sync.dma_start(idt[:], idt_d.ap())

            def mm(out, lhsT, rhs):
                nc.tensor.matmul(
                    out,
                    lhsT.bitcast(MM_DT),
                    rhs.bitcast(MM_DT),
                    start=True,
                    stop=True,
                )

            def recip1p(dst, src):
                """dst = 1 / (1 + src), via exp(-ln(1+src)) on ScalarE."""
                t = state.tile([128, SFREE], F32, tag="lntmp")
                nc.scalar.activation(t[:], src, AFT.Ln, bias=1.0)
                nc.scalar.activation(dst, t[:], AFT.Exp, scale=-1.0)

            for c in range(n_chunks):
                # ---- load + transpose to grouped layout ----
                at_bm = io.tile([128, SFREE], F32, tag="at_bm")
                nc.sync.dma_start(at_bm[:], at_chunks[c])

                at_g = io.tile([128, SFREE], F32, tag="at_g")
                for s in range(4):
                    tp = tps.tile([128, 128], F32, tag="tpsum")
                    nc.tensor.transpose(
                        tp[:], at_bm[:, s * 128:(s + 1) * 128], idt[:]
                    )
                    nc.scalar.copy(at_g[:, s * 128:(s + 1) * 128], tp[:])

                # ---- fixed-point iterations ----
                bf = state.tile([128, SFREE], F32, tag="bf")
                nc.gpsimd.memset(bf[:], 1.0)

                af = None
                for t in range(n_iters):
                    u_ps = uvps.tile([128, SFREE], F32, tag="uv_ps")
                    mm(u_ps[:], wu[:], bf[:])
                    ru = state.tile([128, SFREE], F32, tag="ru")
                    recip1p(ru[:], u_ps[:])
                    af = state.tile([128, SFREE], F32, tag="af")
                    nc.vector.tensor_mul(af[:], at_g[:], ru[:])

                    v_ps = uvps.tile([128, SFREE], F32, tag="uv_ps")
                    mm(v_ps[:], wv[:], af[:])
                    bf = state.tile([128, SFREE], F32, tag="bf")
                    recip1p(bf[:], v_ps[:])

                # ---- final correction half-step: AF_ from final BF ----
                u_ps = uvps.tile([128, SFREE], F32, tag="uv_ps")
                mm(u_ps[:], wu[:], bf[:])
                ru = state.tile([128, SFREE], F32, tag="ru")
                recip1p(ru[:], u_ps[:])
                af_ = state.tile([128, SFREE], F32, tag="af")
                nc.vector.tensor_mul(af_[:], at_g[:], ru[:])

                # ---- outer-product output stage ----
                c_sb = io.tile([128, 64 * 64], F32, tag="c_sb")
                for s in range(4):
                    for h in range(2):
                        d_ps = deps.tile([128, 512], F32, tag="de_ps")
                        mm(
                            d_ps[:],
                            af_[:, s * 128:(s + 1) * 128],
                            w2d[:, h * 512:(h + 1) * 512],
                        )
                        e_ps = deps.tile([128, 512], F32, tag="de_ps")
                        mm(
                            e_ps[:],
                            bf[:, s * 128:(s + 1) * 128],
                            w2e[:, h * 512:(h + 1) * 512],
                        )
                        # TensorTensor may read at most one PSUM operand:
                        # stage E through SBUF on the Scalar engine.
                        e_sb = state.tile([128, 512], F32, tag="e_sb")
                        nc.scalar.copy(e_sb[:], e_ps[:])
                        off = s * 1024 + h * 512
                        nc.vector.tensor_mul(
                            c_sb[:, off:off + 512], d_ps[:], e_sb[:]
                        )

                nc.sync.dma_start(c_chunks[c], c_sb[:])

    nc.compile()
    return nc


@functools.lru_cache(maxsize=2)
def _get_program(rows_per_core: int, n_iters: int):
    return _build_program(rows_per_core, n_iters)


def kernel(AT: np.ndarray, k: np.ndarray, bt: np.ndarray) -> np.ndarray:
    AT = np.ascontiguousarray(AT, np.float32)
    k = np.asarray(k, np.float32)
    bt = np.asarray(bt, np.float32)
    B = AT.shape[0]
    rows = B // N_CORES

    wts = _host_weights(k, bt)
    nc = _get_program(rows, N_ITERS)

    in_maps = []
    for c in range(N_CORES):
        m = {"AT": AT[c * rows:(c + 1) * rows]}
        m.update(wts)
        in_maps.append(m)

    res = bass_utils.run_bass_kernel_spmd(nc, in_maps, core_ids=list(range(N_CORES)))
    C = np.concatenate([r["C"] for r in res.results], axis=0)
    return C.reshape(B, NA, NB).astype(np.float32)


if __name__ == "__main__":
    # smoke test with random data
    rng = np.random.default_rng(0)
    AT = rng.random((B_TOTAL, NA), np.float32)
    k = rng.standard_normal((NA, NB)).astype(np.float32)
    bt = np.zeros((1, NB), np.float32)
    C = kernel(AT=AT, k=k, bt=bt)
    print("C", C.shape, C.dtype, float(np.abs(C).max()))
